# revision 12
# baseline (speedup 1.0000x reference)
"""Trainium2 Bass kernel for attention with softmax over the *query* axis.

Reference computation (B=2, N=8192, D=256, fp32):
    Q = x @ Wq.T ; K = x @ Wk.T ; V = x @ Wv.T          # [B, N, D]
    s = Q @ K.T / sqrt(D)                                # [B, N, N]
    attn = softmax(s, axis=1)       # softmax over the QUERY axis
    out = attn @ V                                       # [B, N, D]

Sharding: 4 cores per batch, each owning a 2048-key chunk.  Softmax over
the query axis makes Z[k] = sum_q exp(s[q,k]) a per-key reduction, so a
key shard keeps the softmax fully local; the host adds the per-core
output partials.

Per-core restructuring (keys on partitions):
    A' = Wk.T @ Wq                 [D, D]
    G  = A'.T @ x_c.T              [D, 2048]   (key side folded first --
                                    4x cheaper than the query-side fold)
    sT[k, q] = (G.T x.T)[k, q]
    E  = exp(sT / sqrt(D))         (ACT, accum_out -> Z[k]; |s/sqrt(D)|
                                    is < ~3 so no max-subtraction needed)
    outT_partial = (V / Z).T @ E   [D, N]

Implementation notes:
  * x is cast to bf16 on the HOST and uploaded pre-rotated per core so
    the core's keys are always rows [0, 2048).  x^T is produced by XBAR
    DMA-transposes straight from the bf16 input into PER-CHUNK tiles, so
    the first scores only wait for the first chunk (short head).
  * E is produced in 256-key sub-chunks (n_sub=8).  Pass 2 consumes
    groups (0,1), (2,3), (4,5), (6), (7): paired groups amortize partial
    writes; the final single-sub groups minimize the post-exp tail,
    which is gated by the last sub's Z reduction.
  * Pass-2 matmul units for a group are emitted interleaved into the
    pass-1 stream of later subs (V-projection units fill sub 0) so the
    PE never idles while the scalar engine runs the exp chain (HAM
    stays at full clock).
  * Matmul loops keep the stationary operand fixed across consecutive
    matmuls so LDWEIGHTS amortizes.
"""

import functools

import numpy as np

# ---- problem constants (hardcoded per the harness contract) ----
B = 2
N = 8192
D = 256
N_CORES = 8
CORES_PER_BATCH = N_CORES // B
CHUNK = N // CORES_PER_BATCH          # 2048 keys per core
N_SUB = 8                             # pass-1 sub-chunks per core
GROUPS = ((0, 1), (2, 3), (4, 5), (6,), (7,))   # pass-2 output groups
SCALE = 1.0 / 16.0                    # 1/sqrt(D)


def _build_program(n=N, chunk=CHUNK, n_sub=N_SUB, n_devices=N_CORES,
                   enable_asserts=False):
    import concourse.bass as bass
    import concourse.tile as tile
    from concourse import bacc, mybir
    from concourse.masks import make_identity

    f32 = mybir.dt.float32
    f16 = mybir.dt.float16
    bf16 = mybir.dt.bfloat16
    ts = bass.ts
    P = 128

    n_kt = chunk // P             # key tiles per core (16)
    kq = n_kt // n_sub            # key tiles per sub-chunk (2)
    nqg = n // 1024               # 1024-wide query tiles (8)

    nc = bacc.Bacc("TRN2", target_bir_lowering=False, debug=False,
                   enable_asserts=enable_asserts, num_devices=n_devices)

    xb = nc.dram_tensor("xb", [n, D], bf16, kind="ExternalInput").ap()
    wq = nc.dram_tensor("wq", [D, D], f32, kind="ExternalInput").ap()
    wk = nc.dram_tensor("wk", [D, D], f32, kind="ExternalInput").ap()
    wv = nc.dram_tensor("wv", [D, D], f32, kind="ExternalInput").ap()
    out_part = nc.dram_tensor("out_part", [len(GROUPS), 2, P, n], f16,
                              kind="ExternalOutput").ap()

    Exp = mybir.ActivationFunctionType.Exp

    with tile.TileContext(nc) as tc:
        with (
            tc.tile_pool(name="const", bufs=1) as const_pool,
            tc.tile_pool(name="proj", bufs=1) as proj_pool,
            tc.tile_pool(name="xq", bufs=1) as xq_pool,
            tc.tile_pool(name="vpool", bufs=1) as v_pool,
            tc.tile_pool(name="gpool", bufs=1) as g_pool,
        ):
            ident = const_pool.tile([P, P], f32)
            make_identity(nc, ident[:])

            A_sb = proj_pool.tile([P, 2, D], bf16)     # A'[d', d]
            WvT_sb = proj_pool.tile([P, 2, D], bf16)   # Wv.T[d, j]
            V_sb = v_pool.tile([P, n_kt, D], bf16)     # V[k, j] (k tiles)
            G_sb = g_pool.tile([P, 2, chunk], bf16)    # G[d, k]
            # x^T in per-chunk tiles so consumers only wait on the chunks
            # they read (chunks 0,1 are this core's keys -- pre-rotated).
            xq_t = [xq_pool.tile([P, 2, 1024], bf16, name=f"xq{qc}",
                                 tag=f"xq{qc}")
                    for qc in range(nqg)]

            def xkey(kt, dh):
                """Stationary slice of the key transpose for key tile kt."""
                return xq_t[kt // 8][:, dh, ts(kt % 8, P)]

            # ---------------- phase A: transposes + projections ----------
            with (
                tc.tile_pool(name="wstage", bufs=1) as wstage,
                tc.tile_pool(name="psA", bufs=2, space="PSUM") as psA,
                tc.tile_pool(name="psT", bufs=2, space="PSUM") as psT,
                tc.tile_pool(name="psG", bufs=2, space="PSUM") as psG,
            ):
                wq_sb = wstage.tile([P, 2, D], f32)
                wk_sb = wstage.tile([P, 2, D], f32)
                wv_sb = wstage.tile([P, 2, D], f32)
                nc.sync.dma_start(wq_sb[:], wq.rearrange("(c p) d -> p c d", p=P))
                nc.sync.dma_start(wk_sb[:], wk.rearrange("(c p) d -> p c d", p=P))
                nc.sync.dma_start(wv_sb[:], wv.rearrange("(c p) d -> p c d", p=P))

                for qc in range(nqg):
                    for dh in range(2):
                        nc.sync.dma_start(out=xq_t[qc][:, dh, :],
                                          in_=xb[ts(qc, 1024), ts(dh, P)],
                                          transpose=True)

                # A'[d', d] = sum_i Wk[i, d'] * Wq[i, d]
                for dh in range(2):
                    aps = psA.tile([P, D], f32, tag="ps")
                    for ic in range(2):
                        nc.tensor.matmul(aps[:], wk_sb[:, ic, ts(dh, P)],
                                         wq_sb[:, ic, :],
                                         start=(ic == 0), stop=(ic == 1))
                    nc.vector.tensor_copy(A_sb[:, dh, :], aps[:])

                # Wv.T[d, j]
                for ic in range(2):
                    for dh in range(2):
                        tps = psT.tile([P, P], f32)
                        nc.tensor.transpose(tps[:], wv_sb[:, ic, ts(dh, P)],
                                            ident[:])
                        nc.vector.tensor_copy(WvT_sb[:, dh, ts(ic, P)], tps[:])

                # G[d, k] = sum_d' A'[d', d] * x_c[k, d']
                for dt in range(2):
                    for kh in range(2):
                        gps = psG.tile([P, 1024], f32)
                        for dh in range(2):
                            for ks in range(2):
                                nc.tensor.matmul(
                                    gps[:, ts(ks, 512)],
                                    A_sb[:, dh, ts(dt, P)],
                                    xq_t[kh][:, dh, ts(ks, 512)],
                                    start=(dh == 0), stop=(dh == 1))
                        nc.vector.tensor_copy(G_sb[:, dt, ts(kh, 1024)],
                                              gps[:])

            # ---------------- main loop over key sub-chunks ----------------
            with (
                tc.tile_pool(name="epool", bufs=4) as e_pool,
                tc.tile_pool(name="zpool", bufs=2) as z_pool,
                tc.tile_pool(name="vp", bufs=4) as vp_pool,
                tc.tile_pool(name="ostage", bufs=3) as o_pool,
                tc.tile_pool(name="psS", bufs=2, space="PSUM") as psS,
                tc.tile_pool(name="psO", bufs=2, space="PSUM") as psO,
            ):
                E_gen = [None] * n_sub
                Vp_gen = [None] * n_sub

                def v_unit(kt):
                    """V[k, j] = sum_d x_c[k, d] * Wv[j, d] for one k tile."""
                    # same shape/tag as the pass-2 tiles so psO stays 4 banks
                    vps = psO.tile([P, 1024], f32, tag="ops")
                    for dh in range(2):
                        nc.tensor.matmul(vps[:, :D], xkey(kt, dh),
                                         WvT_sb[:, dh, :],
                                         start=(dh == 0), stop=(dh == 1))
                    nc.vector.tensor_copy(V_sb[:, kt, :], vps[:, :D])

                def pass2_unit(gi, qg, j):
                    """out_part[gi, j, :, qg*1024:...] accumulation."""
                    subs = GROUPS[gi]
                    ops = psO.tile([P, 1024], f32, tag="ops")
                    first = True
                    for si in subs:
                        Es, Vs = E_gen[si], Vp_gen[si]
                        for kt in range(kq):
                            last = (si == subs[-1] and kt == kq - 1)
                            for qb in range(2):
                                nc.tensor.matmul(
                                    ops[:, ts(qb, 512)],
                                    Vs[:, kt, ts(j, P)],
                                    Es[:, kt, ts(qg * 2 + qb, 512)],
                                    start=first, stop=last)
                            first = False
                    ost = o_pool.tile([P, 1024], f16)
                    nc.vector.tensor_copy(ost[:], ops[:])
                    nc.sync.dma_start(out_part[gi, j, :, ts(qg, 1024)],
                                      ost[:])

                def group_units(gi):
                    return [("p2", gi, qg, j)
                            for qg in range(nqg) for j in range(2)]

                def spread(units):
                    """Spread <=8 units over 16 slots (odd positions)."""
                    sched = [None] * 16
                    for i, u in enumerate(units):
                        sched[2 * i + 1] = u
                    return sched

                # per-sub fill schedule: V projections during subs 0-1,
                # pair group g during subs 2g+2 / 2g+3, single-sub group 3
                # (sub 6) densely during sub 7, group 4 (sub 7) as tail.
                schedule = [[None] * 16 for _ in range(n_sub)]
                schedule[0] = spread([("v", kt) for kt in range(8)])
                schedule[1] = spread([("v", kt) for kt in range(8, 16)])
                for g in range(2):
                    u = group_units(g)
                    schedule[2 * g + 2] = spread(u[:8])
                    schedule[2 * g + 3] = spread(u[8:])
                schedule[6] = group_units(2)        # 16 slots, every unit
                schedule[7] = group_units(3)        # 16 slots, every unit
                tail = group_units(4)

                def emit(u):
                    if u is None:
                        return
                    if u[0] == "v":
                        v_unit(u[1])
                    else:
                        pass2_unit(*u[1:])

                for sub in range(n_sub):
                    E_t = e_pool.tile([P, kq, n], bf16)
                    E_gen[sub] = E_t
                    Zp = z_pool.tile([P, kq, nqg], f32)

                    # pass 1: scores -> exp -> E (+ Z partials), with fill
                    # units interleaved to keep the PE busy while the scalar
                    # engine runs the exp chain.
                    unit = 0
                    for kt in range(kq):
                        ktg = sub * kq + kt
                        for qg in range(nqg):
                            sps = psS.tile([P, 1024], f32)
                            for dh in range(2):
                                for qb in range(2):
                                    nc.tensor.matmul(
                                        sps[:, ts(qb, 512)],
                                        G_sb[:, dh, ts(ktg, P)],
                                        xq_t[qg][:, dh, ts(qb, 512)],
                                        start=(dh == 0), stop=(dh == 1))
                            nc.scalar.activation(
                                E_t[:, kt, ts(qg, 1024)], sps[:], Exp,
                                scale=SCALE,
                                accum_out=Zp[:, kt, qg:qg + 1])
                            emit(schedule[sub][unit])
                            unit += 1

                    # finalize Z, fold 1/Z into V
                    Z = z_pool.tile([P, kq], f32)
                    nc.vector.tensor_reduce(
                        Z[:], Zp[:],
                        axis=mybir.AxisListType.X, op=mybir.AluOpType.add)
                    rz = z_pool.tile([P, kq], f32)
                    nc.vector.reciprocal(rz[:], Z[:])
                    Vp = vp_pool.tile([P, kq, D], bf16)
                    Vp_gen[sub] = Vp
                    for kt in range(kq):
                        nc.vector.tensor_scalar_mul(
                            Vp[:, kt, :], V_sb[:, sub * kq + kt, :],
                            rz[:, kt:kt + 1])

                for u in tail:
                    emit(u)

    nc.compile()
    return nc


@functools.lru_cache(maxsize=1)
def _get_compiled():
    return _build_program()


def kernel(x, Wq, Wk, Wv):
    import ml_dtypes
    from concourse.bass_utils import run_bass_kernel_spmd

    nc = _get_compiled()

    x = np.ascontiguousarray(x, dtype=np.float32)
    xbf = x.astype(ml_dtypes.bfloat16)
    wq = np.ascontiguousarray(Wq, dtype=np.float32)
    wk = np.ascontiguousarray(Wk, dtype=np.float32)
    wv = np.ascontiguousarray(Wv, dtype=np.float32)

    in_maps = []
    for c in range(N_CORES):
        b = c // CORES_PER_BATCH
        k0 = (c % CORES_PER_BATCH) * CHUNK
        in_maps.append({
            "xb": np.ascontiguousarray(np.roll(xbf[b], -k0, axis=0)),
            "wq": wq,
            "wk": wk,
            "wv": wv,
        })

    res = run_bass_kernel_spmd(nc, in_maps, list(range(N_CORES)))
    global LAST_RESULTS, LAST_EXEC_TIME_NS
    LAST_RESULTS = res
    LAST_EXEC_TIME_NS = res.exec_time_ns

    out = np.empty((B, N, D), dtype=np.float32)
    for b in range(B):
        acc = np.zeros((N, D), dtype=np.float32)
        for c in range(b * CORES_PER_BATCH, (b + 1) * CORES_PER_BATCH):
            k0 = (c % CORES_PER_BATCH) * CHUNK
            p = res.results[c]["out_part"].astype(np.float32)   # [5,2,128,n]
            pT = p.sum(axis=0).reshape(D, N).T                  # [n(q-rot), D]
            acc += np.roll(pT, k0, axis=0)
        out[b] = acc
    return out


# revision 16
# speedup vs baseline: 1.1498x; 1.1498x over previous
"""Trainium2 Bass kernel for attention with softmax over the *query* axis.

Reference computation (B=2, N=8192, D=256, fp32):
    Q = x @ Wq.T ; K = x @ Wk.T ; V = x @ Wv.T          # [B, N, D]
    s = Q @ K.T / sqrt(D)                                # [B, N, N]
    attn = softmax(s, axis=1)       # softmax over the QUERY axis
    out = attn @ V                                       # [B, N, D]

Sharding: 4 cores per batch, each owning a 2048-key chunk.  Softmax over
the query axis makes Z[k] = sum_q exp(s[q,k]) a per-key reduction, so a
key shard keeps the softmax fully local; the host adds the per-core
output partials.

Per-core restructuring (keys on partitions):
    A' = Wk.T @ Wq                 [D, D]
    G  = A'.T @ x_c.T              [D, 2048]   (key side folded first --
                                    4x cheaper than the query-side fold)
    sT[k, q] = (G.T x.T)[k, q]
    E  = exp(sT / sqrt(D))         (ACT, accum_out -> Z[k]; |s/sqrt(D)|
                                    is < ~3 so no max-subtraction needed)
    outT_partial = V''.T @ E       [D, N],  V'' = V * 8192/Z  (the 2^13
                                    scale keeps V'' in fp8 normal range;
                                    the host divides it back out)

Pass 2 runs in fp8 (e4m3) with MatmulPerfMode.DoubleRow (2 fp8 MACs per
PE cell per cycle): E is written as fp8 straight from the ACT, V'' is
quantized by the DVE.  The dominant fp8 error terms are compensated with
per-j column sums: out += (c_j - s8_j) where c_j = sum_k V''[k,j]
(unquantized, via an rz-weighted matmul on bf16 V) and s8_j =
sum_k fp8(V'')[k,j] (ones-matmul).  This removes the constant-E
component of the V-quantization error; measured rel err ~1.4e-2 against
a 2e-2 gate.

Other implementation notes:
  * x is cast to bf16 on the HOST and uploaded pre-rotated per core so
    the core's keys are always rows [0, 2048).  x^T is produced by XBAR
    DMA-transposes straight from the bf16 input into PER-CHUNK tiles.
  * E is produced in 256-key sub-chunks (n_sub=8); pass 2 consumes
    PAIRS of sub-chunks, writing fp16 partials once per 512 keys.
  * Pass-2 matmul units for pair m are emitted interleaved into the
    pass-1 stream of subs 2m+2 and 2m+3 (V-projection units fill sub 0)
    so the PE never idles while the scalar engine runs the exp chain.
"""

import functools

import numpy as np

# ---- problem constants (hardcoded per the harness contract) ----
B = 2
N = 8192
D = 256
N_CORES = 8
CORES_PER_BATCH = N_CORES // B
CHUNK = N // CORES_PER_BATCH          # 2048 keys per core
N_SUB = 8                             # pass-1 sub-chunks per core
N_PAIR = N_SUB // 2                   # pass-2 works on sub-chunk pairs
SCALE = 1.0 / 16.0                    # 1/sqrt(D)
VS = 8192.0                           # V'' scale (2^13), divided out on host


def _build_program(n=N, chunk=CHUNK, n_sub=N_SUB, n_devices=N_CORES,
                   enable_asserts=False):
    import concourse.bass as bass
    import concourse.tile as tile
    from concourse import bacc, mybir
    from concourse.masks import make_identity

    f32 = mybir.dt.float32
    f16 = mybir.dt.float16
    bf16 = mybir.dt.bfloat16
    fp8 = mybir.dt.float8e4
    ts = bass.ts
    P = 128
    DR = mybir.MatmulPerfMode.DoubleRow

    n_kt = chunk // P             # key tiles per core (16)
    kq = n_kt // n_sub            # key tiles per sub-chunk (2)
    nqg = n // 1024               # 1024-wide query tiles (8)
    nqb = n // 512                # 512-wide query tiles (16)

    nc = bacc.Bacc("TRN2", target_bir_lowering=False, debug=False,
                   enable_asserts=enable_asserts, num_devices=n_devices)

    xb = nc.dram_tensor("xb", [n, D], bf16, kind="ExternalInput").ap()
    wq = nc.dram_tensor("wq", [D, D], f32, kind="ExternalInput").ap()
    wk = nc.dram_tensor("wk", [D, D], f32, kind="ExternalInput").ap()
    wv = nc.dram_tensor("wv", [D, D], f32, kind="ExternalInput").ap()
    out_part = nc.dram_tensor("out_part", [N_PAIR, 2, P, n], f16,
                              kind="ExternalOutput").ap()

    Exp = mybir.ActivationFunctionType.Exp
    Ident = mybir.ActivationFunctionType.Identity

    with tile.TileContext(nc) as tc:
        with (
            tc.tile_pool(name="const", bufs=1) as const_pool,
            tc.tile_pool(name="proj", bufs=1) as proj_pool,
            tc.tile_pool(name="xq", bufs=1) as xq_pool,
            tc.tile_pool(name="vpool", bufs=1) as v_pool,
            tc.tile_pool(name="gpool", bufs=1) as g_pool,
        ):
            ident = const_pool.tile([P, P], f32)
            make_identity(nc, ident[:])
            ones8 = const_pool.tile([P, 1], fp8)
            nc.vector.memset(ones8[:], 1.0)

            A_sb = proj_pool.tile([P, 2, D], bf16)     # A'[d', d]
            WvT_sb = proj_pool.tile([P, 2, D], bf16)   # Wv.T[d, j]
            V_sb = v_pool.tile([P, n_kt, D], bf16)     # V[k, j] (k tiles)
            G_sb = g_pool.tile([P, 2, chunk], bf16)    # G[d, k]
            # x^T in per-chunk tiles so consumers only wait on the chunks
            # they read (chunks 0,1 are this core's keys -- pre-rotated).
            xq_t = [xq_pool.tile([P, 2, 1024], bf16, name=f"xq{qc}",
                                 tag=f"xq{qc}")
                    for qc in range(nqg)]

            def xkey(kt, dh):
                """Stationary slice of the key transpose for key tile kt."""
                return xq_t[kt // 8][:, dh, ts(kt % 8, P)]

            # ---------------- phase A: transposes + projections ----------
            with (
                tc.tile_pool(name="wstage", bufs=1) as wstage,
                tc.tile_pool(name="psA", bufs=2, space="PSUM") as psA,
                tc.tile_pool(name="psT", bufs=2, space="PSUM") as psT,
                tc.tile_pool(name="psG", bufs=2, space="PSUM") as psG,
            ):
                wq_sb = wstage.tile([P, 2, D], f32)
                wk_sb = wstage.tile([P, 2, D], f32)
                wv_sb = wstage.tile([P, 2, D], f32)

                # key-chunk transposes first (everything needs them), then
                # the weights, then the remaining query chunks.
                for qc in range(2):
                    for dh in range(2):
                        nc.sync.dma_start(out=xq_t[qc][:, dh, :],
                                          in_=xb[ts(qc, 1024), ts(dh, P)],
                                          transpose=True)
                nc.sync.dma_start(wq_sb[:], wq.rearrange("(c p) d -> p c d", p=P))
                nc.sync.dma_start(wk_sb[:], wk.rearrange("(c p) d -> p c d", p=P))
                nc.sync.dma_start(wv_sb[:], wv.rearrange("(c p) d -> p c d", p=P))
                for qc in range(2, nqg):
                    for dh in range(2):
                        nc.sync.dma_start(out=xq_t[qc][:, dh, :],
                                          in_=xb[ts(qc, 1024), ts(dh, P)],
                                          transpose=True)

                # A'[d', d] = sum_i Wk[i, d'] * Wq[i, d]
                for dh in range(2):
                    aps = psA.tile([P, D], f32, tag="ps")
                    for ic in range(2):
                        nc.tensor.matmul(aps[:], wk_sb[:, ic, ts(dh, P)],
                                         wq_sb[:, ic, :],
                                         start=(ic == 0), stop=(ic == 1))
                    nc.vector.tensor_copy(A_sb[:, dh, :], aps[:])

                # Wv.T[d, j]
                for ic in range(2):
                    for dh in range(2):
                        tps = psT.tile([P, P], f32)
                        nc.tensor.transpose(tps[:], wv_sb[:, ic, ts(dh, P)],
                                            ident[:])
                        nc.vector.tensor_copy(WvT_sb[:, dh, ts(ic, P)], tps[:])

                # G[d, k] = sum_d' A'[d', d] * x_c[k, d']
                for kh in range(2):
                    for dt in range(2):
                        gps = psG.tile([P, 1024], f32)
                        for dh in range(2):
                            for ks in range(2):
                                nc.tensor.matmul(
                                    gps[:, ts(ks, 512)],
                                    A_sb[:, dh, ts(dt, P)],
                                    xq_t[kh][:, dh, ts(ks, 512)],
                                    start=(dh == 0), stop=(dh == 1))
                        nc.vector.tensor_copy(G_sb[:, dt, ts(kh, 1024)],
                                              gps[:])

            # ---------------- main loop over key sub-chunks ----------------
            with (
                tc.tile_pool(name="epool", bufs=4) as e_pool,
                tc.tile_pool(name="zpool", bufs=2) as z_pool,
                tc.tile_pool(name="vp", bufs=4) as vp_pool,
                tc.tile_pool(name="ostage", bufs=4) as o_pool,
                tc.tile_pool(name="psS", bufs=2, space="PSUM") as psS,
                tc.tile_pool(name="psO", bufs=2, space="PSUM") as psO,
                tc.tile_pool(name="psC", bufs=1, space="PSUM") as psC,
            ):
                E_gen = [None] * n_sub
                Vp_gen = [None] * n_sub
                # packed column-sum accumulators: [c_j0, c_j1, s8_j0, s8_j1]
                cps = psC.tile([P, 4], f32)

                def v_unit(kt):
                    """V[k, j] = sum_d x_c[k, d] * Wv[j, d] for one k tile."""
                    vps = psO.tile([P, 512], f32, tag="ops")
                    for dh in range(2):
                        nc.tensor.matmul(vps[:, :D], xkey(kt, dh),
                                         WvT_sb[:, dh, :],
                                         start=(dh == 0), stop=(dh == 1))
                    nc.vector.tensor_copy(V_sb[:, kt, :], vps[:, :D])

                def pass2_unit(pair, qb, j, corr=None, copy_eng=None):
                    """out_part[pair, j, :, qb*512:...] via fp8 DoubleRow."""
                    subs = (2 * pair, 2 * pair + 1)
                    ops = psO.tile([P, 512], f32, tag="ops")
                    for i, si in enumerate(subs):
                        nc.tensor.matmul(
                            ops[:], Vp_gen[si][:, :, ts(j, P)],
                            E_gen[si][:, :, ts(qb, 512)],
                            start=(i == 0), stop=(i == 1),
                            perf_mode=DR)
                    ost = o_pool.tile([P, 512], f16)
                    if corr is None:
                        nc.vector.tensor_copy(ost[:], ops[:])
                    elif copy_eng == "scalar":
                        nc.scalar.activation(ost[:], ops[:], Ident,
                                             bias=corr)
                    else:
                        nc.vector.tensor_scalar_add(ost[:], ops[:], corr)
                    nc.sync.dma_start(out_part[pair, j, :, ts(qb, 512)],
                                      ost[:])

                fill = [("v", kt) for kt in range(n_kt)]
                fill_rate = 1          # units per pass-1 step (subs 0-1)

                def emit_fill():
                    for _ in range(fill_rate):
                        if not fill:
                            return
                        u = fill.pop(0)
                        if u[0] == "v":
                            v_unit(u[1])
                        else:
                            pass2_unit(*u[1:])

                for sub in range(n_sub):
                    E_t = e_pool.tile([P, kq, n], fp8)
                    E_gen[sub] = E_t
                    Zp = z_pool.tile([P, kq, nqg], f32)

                    # pass 1: scores -> exp -> E (+ Z partials), with fill
                    # units interleaved to keep the PE busy while the scalar
                    # engine runs the exp chain.
                    unit = 0
                    for kt in range(kq):
                        ktg = sub * kq + kt
                        for qg in range(nqg):
                            sps = psS.tile([P, 1024], f32)
                            for dh in range(2):
                                for qb in range(2):
                                    nc.tensor.matmul(
                                        sps[:, ts(qb, 512)],
                                        G_sb[:, dh, ts(ktg, P)],
                                        xq_t[qg][:, dh, ts(qb, 512)],
                                        start=(dh == 0), stop=(dh == 1))
                            nc.scalar.activation(
                                E_t[:, kt, ts(qg, 1024)], sps[:], Exp,
                                scale=SCALE,
                                accum_out=Zp[:, kt, qg:qg + 1])
                            if unit % 2 == 1:
                                emit_fill()
                            unit += 1

                    # finalize Z; V'' = V * (VS/Z) quantized to fp8
                    Z = z_pool.tile([P, kq], f32)
                    nc.vector.tensor_reduce(
                        Z[:], Zp[:],
                        axis=mybir.AxisListType.X, op=mybir.AluOpType.add)
                    Zs = z_pool.tile([P, kq], f32)
                    nc.vector.tensor_scalar_mul(Zs[:], Z[:], 1.0 / VS)
                    rz = z_pool.tile([P, kq], f32)
                    nc.vector.reciprocal(rz[:], Zs[:])
                    rzb = z_pool.tile([P, kq], bf16)
                    nc.vector.tensor_copy(rzb[:], rz[:])
                    Vp = vp_pool.tile([P, kq, D], fp8)
                    Vp_gen[sub] = Vp
                    for kt in range(kq):
                        nc.vector.tensor_scalar_mul(
                            Vp[:, kt, :], V_sb[:, sub * kq + kt, :],
                            rz[:, kt:kt + 1])

                    # column-sum correction accumulators:
                    #   c_j  += V[:,j].T @ rz   (unquantized V'')
                    #   s8_j += V8[:,j].T @ 1   (quantized V'')
                    # NOTE: start=True pending-zeroes the whole 2KB PSUM
                    # bank (ZERO_REGION_SIZE), so only the very first matmul
                    # may set it -- the other columns' first writes consume
                    # the same bank-wide pending-zero marks.
                    for kt in range(kq):
                        ktg = sub * kq + kt
                        sp = (ktg == n_kt - 1)
                        for j in range(2):
                            nc.tensor.matmul(
                                cps[:, j:j + 1],
                                V_sb[:, ktg, ts(j, P)], rzb[:, kt:kt + 1],
                                start=(ktg == 0 and j == 0), stop=sp,
                                skip_group_check=True)
                            nc.tensor.matmul(
                                cps[:, 2 + j:3 + j],
                                Vp[:, kt, ts(j, P)], ones8[:],
                                start=False, stop=sp,
                                skip_group_check=True)

                    if sub % 2 == 1:
                        pair = sub // 2
                        fill.extend(("p2", pair, qb, j)
                                    for qb in range(nqb) for j in range(2))
                        fill_rate = 2   # 32 units over the next 16 slots

                # corr[j] = c_j - s8_j, applied to the tail group's copies
                c_sb = z_pool.tile([P, 4], f32)
                nc.vector.tensor_copy(c_sb[:], cps[:])
                corr = z_pool.tile([P, 2], f32)
                nc.vector.tensor_tensor(corr[:], c_sb[:, 0:2], c_sb[:, 2:4],
                                        mybir.AluOpType.subtract)

                # drain the last pair's pass-2 units, alternating the copy
                # engine (the scalar engine is idle during the tail)
                import os
                use_corr = os.environ.get("KERNEL_NO_CORR", "0") != "1"
                for i, u in enumerate(fill):
                    pass2_unit(*u[1:],
                               corr=(corr[:, u[3]:u[3] + 1] if use_corr
                                     else None),
                               copy_eng="scalar" if i % 2 else "vector")

    nc.compile()
    return nc


@functools.lru_cache(maxsize=1)
def _get_compiled():
    return _build_program()


def kernel(x, Wq, Wk, Wv):
    import ml_dtypes
    from concourse.bass_utils import run_bass_kernel_spmd

    nc = _get_compiled()

    x = np.ascontiguousarray(x, dtype=np.float32)
    xbf = x.astype(ml_dtypes.bfloat16)
    wq = np.ascontiguousarray(Wq, dtype=np.float32)
    wk = np.ascontiguousarray(Wk, dtype=np.float32)
    wv = np.ascontiguousarray(Wv, dtype=np.float32)

    in_maps = []
    for c in range(N_CORES):
        b = c // CORES_PER_BATCH
        k0 = (c % CORES_PER_BATCH) * CHUNK
        in_maps.append({
            "xb": np.ascontiguousarray(np.roll(xbf[b], -k0, axis=0)),
            "wq": wq,
            "wk": wk,
            "wv": wv,
        })

    res = run_bass_kernel_spmd(nc, in_maps, list(range(N_CORES)))
    global LAST_RESULTS, LAST_EXEC_TIME_NS
    LAST_RESULTS = res
    LAST_EXEC_TIME_NS = res.exec_time_ns

    out = np.empty((B, N, D), dtype=np.float32)
    for b in range(B):
        acc = np.zeros((N, D), dtype=np.float32)
        for c in range(b * CORES_PER_BATCH, (b + 1) * CORES_PER_BATCH):
            k0 = (c % CORES_PER_BATCH) * CHUNK
            p = res.results[c]["out_part"].astype(np.float32)   # [4,2,128,n]
            pT = p.sum(axis=0).reshape(D, N).T                  # [n(q-rot), D]
            acc += np.roll(pT, k0, axis=0)
        out[b] = acc * np.float32(1.0 / VS)
    return out
